# revision 33
# baseline (speedup 1.0000x reference)
"""NSVQ vector-quantization forward on 8 Trainium2 NeuronCores.

Data-parallel over tokens: z [32,1024,64] -> 32768 tokens, 4096 per core;
codebook W [8192, 64] replicated.

Per-core device pipeline (per 128-token tile):
  1. bf16 "prune" matmul on TensorE: dn = 2*z.w - ||w||^2 (negated distance,
     argmax == argmin of distance), via an augmented contraction row (K=65).
  2. ScalarE evacuates PSUM -> SBUF bf16 store of scores.
  3. VectorE max8/max_index -> top-8 candidate codes per token.
  4. GPSIMD indirect DMA gathers the 8 candidate codebook rows (fp32).
  5. Exact fp32 rescore sum((z-w)^2) -> winner selection (matches fp32 argmin
     of the reference exactly; validated: min inter-code gap 1.3e-4 >> noise).
  6. z_q, idx, per-token squared-error partials written out.
Host: shard/unshard, tiny final reductions (loss, histogram -> perplexity).
"""

import numpy as np
import ml_dtypes

import concourse.bass as bass
import concourse.tile as tile
import concourse.tile as tilemod
from concourse import mybir
from concourse.tile import ScopedClock
from concourse.bass import IndirectOffsetOnAxis
from concourse.bass_utils import run_bass_kernel_spmd

BF16 = mybir.dt.bfloat16
F32 = mybir.dt.float32
U32 = mybir.dt.uint32
I32 = mybir.dt.int32

N_CORES = 8
D = 64
N_E = 8192
TOK_PER_CORE = 4096
TILE_TOK = 128
N_TILES = TOK_PER_CORE // TILE_TOK  # 32
CHUNK = 512
N_CHUNKS = N_E // CHUNK  # 16
NCAND = 8
BIG = 1.0e4

BETA = 0.25
EPS = 1e-12


def _patched_drain_and_barrier(self, tick_clock, wait_clock):
    # The walrus build in this container only supports ONE sem wait per
    # CTRL/Drain instruction; Tile's stock drain attaches all outstanding
    # DMA-queue waits to a single Drain and codegen fails with "Too many
    # sync wait commands".  Re-emit as a chain of single-wait drains.
    probe = self.nc.sync.drain()
    wait_clock.add_sem_waits(probe.ins, ScopedClock({None: tick_clock.global_clock}))
    waits = list(probe.ins.sync_info.on_wait)
    if len(waits) > 1:
        probe.ins.sync_info = mybir.SyncInfo(on_wait=[waits[0]], on_update=[])
        for w in waits[1:]:
            d = self.nc.sync.drain()
            d.ins.sync_info = mybir.SyncInfo(on_wait=[w], on_update=[])
    self.nc.all_engine_barrier()
    popped = self.nc._tile_sem_poison_stack.pop()
    assert popped is self._sem_poison
    self.nc.clear_and_free_semaphores(list(self.sems.allocated().values()))
    self.nc.all_engine_barrier()


tilemod.TileContext._drain_and_barrier = _patched_drain_and_barrier


_NOP_COUNTER = [0]


def _make_sync_nop(engine, wait=None, update=None):
    _NOP_COUNTER[0] += 1
    nop = mybir.InstDrain(name=f"syncnop-{_NOP_COUNTER[0]}", engine=engine,
                          ins=[], outs=[])
    nop.sync_info = mybir.SyncInfo(
        on_wait=[wait] if wait is not None else [],
        on_update=[update] if update is not None else [])
    return nop


def _redistribute_syncs(insts):
    """The walrus build here supports exactly ONE sem wait and ONE sem update
    per instruction; Tile's scheduler freely attaches several.  Rebalance:

    - surplus WAITS hoist backward onto earlier same-engine instructions that
      carry no wait AND no update, without crossing any update-carrying
      instruction (crossing one could deadlock: a producer may depend on it);
      if that fails, insert a single-wait Drain nop right before.
    - surplus UPDATES get a single-update Drain nop right after.

    Engine program order makes these transforms semantics-preserving.
    Returns the new instruction list.
    """
    def get_si(i):
        return i.sync_info

    def set_si(i, w, u):
        i.sync_info = mybir.SyncInfo(on_wait=w, on_update=u)

    def is_dma(i):
        # async queue-dispatched instructions do not order the sequencer:
        # a wait attached to them does not gate later instructions, and an
        # update moved off them would fire before the transfer lands.
        return "DMA" in type(i).__name__ or "dma" in str(getattr(i, "opcode", "")).lower()

    def is_real(i):
        # Tile meta pseudo-instructions (releases, pool boundaries, branch
        # hints...) are dropped or rewritten during lowering — parking a
        # wait there loses it.
        tn = type(i).__name__
        return tn.startswith("Inst") and "Branch" not in tn

    out = []
    prev_by_engine = {}
    for inst in insts:
        eng = inst.engine
        si = get_si(inst)
        w = list(si.on_wait) if si else []
        u = list(si.on_update) if si else []
        changed = False
        if len(w) > 1:
            surplus = w[:-1]
            for prev in reversed(prev_by_engine.get(eng, [])):
                if not surplus:
                    break
                psi = get_si(prev)
                pw = list(psi.on_wait) if psi else []
                pu = list(psi.on_update) if psi else []
                if pu:
                    break  # never cross an update-carrier
                if not pw and not is_dma(prev) and is_real(prev):
                    set_si(prev, [surplus.pop()], pu)
            for s in surplus:
                out.append(_make_sync_nop(eng, wait=s))
            w = [w[-1]]
            changed = True
        if len(u) > 1:
            if is_dma(inst):
                raise RuntimeError(
                    f"multi-update DMA {inst.name}: cannot split safely")
            changed = True
        if changed:
            set_si(inst, w, [u[0]] if u else [])
        out.append(inst)
        for extra in u[1:]:
            out.append(_make_sync_nop(eng, update=extra))
        prev_by_engine.setdefault(eng, []).append(inst)
    return out


_orig_lower_ordered = tilemod.TileContext._lower_ordered_insts


def _patched_lower_ordered(self, ordered):
    for bb_name in list(ordered.keys()):
        ordered[bb_name] = _redistribute_syncs(ordered[bb_name])
    return _orig_lower_ordered(self, ordered)


tilemod.TileContext._lower_ordered_insts = _patched_lower_ordered


def build_nc(n_tiles=N_TILES):
    nc = bass.Bass("TRN2", target_bir_lowering=False, debug=False,
                   num_devices=N_CORES)

    ntok = n_tiles * TILE_TOK
    z_in = nc.dram_tensor("z", [ntok, D], F32, kind="ExternalInput").ap()
    w_in = nc.dram_tensor("w", [N_E, D], F32, kind="ExternalInput").ap()
    waug_in = nc.dram_tensor("waug", [D + 1, N_E], BF16, kind="ExternalInput").ap()
    ident_in = nc.dram_tensor("ident", [128, 128], F32, kind="ExternalInput").ap()
    iota8_in = nc.dram_tensor("iota8", [128, NCAND], F32, kind="ExternalInput").ap()
    iotab8_in = nc.dram_tensor("iotab8", [128, NCAND], F32, kind="ExternalInput").ap()

    zq_out = nc.dram_tensor("zq", [ntok, D], F32, kind="ExternalOutput").ap()
    idx_out = nc.dram_tensor("idx", [TILE_TOK, n_tiles], I32, kind="ExternalOutput").ap()
    loss_out = nc.dram_tensor("losspart", [TILE_TOK, n_tiles], F32, kind="ExternalOutput").ap()

    with tile.TileContext(nc) as tc:
        with (
            tc.tile_pool(name="consts", bufs=1) as cpool,
            tc.tile_pool(name="waug", bufs=1) as wpool,
            tc.tile_pool(name="acc", bufs=1) as apool,
            tc.tile_pool(name="zin", bufs=3) as zpool,
            tc.tile_pool(name="zt", bufs=2) as ztpool,
            tc.tile_pool(name="dstore", bufs=2) as dpool,
            tc.tile_pool(name="cand", bufs=2) as candpool,
            tc.tile_pool(name="small", bufs=3) as spool,
            tc.tile_pool(name="psT", bufs=2, space="PSUM") as psT,
            tc.tile_pool(name="psMM", bufs=6, space="PSUM") as psMM,
        ):
            ident = cpool.tile([128, 128], F32, tag="ident")
            nc.sync.dma_start(ident[:], ident_in[:])
            iota8 = cpool.tile([128, NCAND], F32, tag="iota8")
            nc.sync.dma_start(iota8[:], iota8_in[:])
            iotab8 = cpool.tile([128, NCAND], F32, tag="iotab8")
            nc.sync.dma_start(iotab8[:], iotab8_in[:])

            waug = wpool.tile([D + 1, N_E], BF16)
            nc.sync.dma_start(waug[:], waug_in[:])

            idx_acc = apool.tile([TILE_TOK, n_tiles], I32, tag="idx_acc")
            loss_acc = apool.tile([TILE_TOK, n_tiles], F32, tag="loss_acc")

            for t in range(n_tiles):
                # -- load z tile and build transposed bf16 lhsT (x2) --------
                z_sb = zpool.tile([TILE_TOK, D], F32)
                nc.sync.dma_start(z_sb[:], z_in[t * TILE_TOK:(t + 1) * TILE_TOK, :])

                ztp = psT.tile([D, TILE_TOK], F32)
                nc.tensor.transpose(ztp[:], z_sb[:], ident[:])

                zt_aug = ztpool.tile([D + 1, TILE_TOK], BF16)
                nc.scalar.mul(zt_aug[0:D, :], ztp[:], 2.0)
                nc.gpsimd.memset(zt_aug[D:D + 1, :], 1.0)

                # -- prune matmuls + evacuation -----------------------------
                d_sb = dpool.tile([TILE_TOK, N_E], BF16)
                for c in range(N_CHUNKS):
                    mm = psMM.tile([TILE_TOK, CHUNK], F32)
                    nc.tensor.matmul(mm[:], zt_aug[:], waug[:, c * CHUNK:(c + 1) * CHUNK],
                                     start=True, stop=True)
                    nc.scalar.copy(d_sb[:, c * CHUNK:(c + 1) * CHUNK], mm[:])

                # -- top-8 candidates ---------------------------------------
                mx8 = spool.tile([TILE_TOK, NCAND], BF16, tag="mx8")
                nc.vector.max(mx8[:], d_sb[:])
                mi_u32 = spool.tile([TILE_TOK, NCAND], U32, tag="mi")
                nc.vector.max_index(mi_u32[:], mx8[:], d_sb[:])

                # -- gather candidate rows (fp32); HW indirect DMA supports
                # one offset per partition, so one call per candidate slot --
                cand = candpool.tile([TILE_TOK, NCAND, D], F32)
                for j in range(NCAND):
                    nc.gpsimd.indirect_dma_start(
                        cand[:, j, :], None, w_in[:],
                        IndirectOffsetOnAxis(ap=mi_u32[:, j:j + 1], axis=0),
                    )

                # -- exact fp32 rescore -------------------------------------
                zb = z_sb[:].unsqueeze(1).broadcast_to((TILE_TOK, NCAND, D))
                diff = candpool.tile([TILE_TOK, NCAND, D], F32, tag="diff")
                nc.vector.tensor_tensor(diff[:], cand[:], zb, mybir.AluOpType.subtract)
                sq = candpool.tile([TILE_TOK, NCAND, D], F32, tag="sq")
                nc.scalar.square(sq[:], diff[:])
                s8 = spool.tile([TILE_TOK, NCAND], F32, tag="s8")
                nc.vector.tensor_reduce(s8[:], sq[:], axis=mybir.AxisListType.X,
                                        op=mybir.AluOpType.add)

                # -- winner selection (first-min tie break) -----------------
                smin = spool.tile([TILE_TOK, 1], F32, tag="smin")
                nc.vector.tensor_reduce(smin[:], s8[:], axis=mybir.AxisListType.X,
                                        op=mybir.AluOpType.min)
                mask8 = spool.tile([TILE_TOK, NCAND], F32, tag="mask8")
                nc.vector.tensor_scalar(mask8[:], s8[:], smin[:], None,
                                        op0=mybir.AluOpType.is_equal)
                t8 = spool.tile([TILE_TOK, NCAND], F32, tag="t8")
                nc.vector.scalar_tensor_tensor(t8[:], mask8[:], -BIG, iotab8[:],
                                               op0=mybir.AluOpType.mult,
                                               op1=mybir.AluOpType.add)
                jstar = spool.tile([TILE_TOK, 1], F32, tag="jstar")
                nc.vector.tensor_reduce(jstar[:], t8[:], axis=mybir.AxisListType.X,
                                        op=mybir.AluOpType.min)
                maskj = spool.tile([TILE_TOK, NCAND], F32, tag="maskj")
                nc.vector.tensor_scalar(maskj[:], iota8[:], jstar[:], None,
                                        op0=mybir.AluOpType.is_equal)

                # final index = sum_j maskj * cand_idx
                mi_f32 = spool.tile([TILE_TOK, NCAND], F32, tag="mif")
                nc.vector.tensor_copy(mi_f32[:], mi_u32[:])
                scr8 = spool.tile([TILE_TOK, NCAND], F32, tag="scr8")
                idx_f = spool.tile([TILE_TOK, 1], F32, tag="idxf")
                nc.vector.tensor_tensor(scr8[:], maskj[:], mi_f32[:],
                                        mybir.AluOpType.mult)
                nc.vector.tensor_reduce(idx_f[:], scr8[:],
                                        axis=mybir.AxisListType.X,
                                        op=mybir.AluOpType.add)
                nc.vector.tensor_copy(idx_acc[:, t:t + 1], idx_f[:])

                # z_q = sum_j maskj * cand  (reduce over j via strided view)
                mjb = maskj[:].unsqueeze(2).broadcast_to((TILE_TOK, NCAND, D))
                zqm = candpool.tile([TILE_TOK, NCAND, D], F32, tag="zqm")
                nc.vector.tensor_tensor(zqm[:], cand[:], mjb, mybir.AluOpType.mult)
                zq_sb = zpool.tile([TILE_TOK, D], F32, tag="zq")
                nc.vector.tensor_reduce(zq_sb[:], zqm[:].rearrange("p j d -> p d j"),
                                        axis=mybir.AxisListType.X,
                                        op=mybir.AluOpType.add)

                # straight-through forward: out = z + (z_q - z); loss partial
                diffw = zpool.tile([TILE_TOK, D], F32, tag="diffw")
                nc.vector.tensor_sub(diffw[:], zq_sb[:], z_sb[:])
                zqo = zpool.tile([TILE_TOK, D], F32, tag="zqo")
                nc.vector.tensor_add(zqo[:], z_sb[:], diffw[:])
                nc.sync.dma_start(zq_out[t * TILE_TOK:(t + 1) * TILE_TOK, :], zqo[:])

                scr64 = zpool.tile([TILE_TOK, D], F32, tag="scr64")
                nc.vector.tensor_tensor(scr64[:], diffw[:], diffw[:],
                                        mybir.AluOpType.mult)
                nc.vector.tensor_reduce(loss_acc[:, t:t + 1], scr64[:],
                                        axis=mybir.AxisListType.X,
                                        op=mybir.AluOpType.add)

            nc.sync.dma_start(idx_out[:], idx_acc[:])
            nc.sync.dma_start(loss_out[:], loss_acc[:])

    return nc


SUBW = 128          # sub-chunk width for hierarchical argmax (V2)
N_SUB = N_E // SUBW  # 64 sub-chunks
NWIN = 4             # top sub-chunks gathered per token (coverage needs 3)
RESC = 4             # candidates rescored in fp32 (coverage needs 3)


def build_nc_v2(n_tiles=N_TILES):
    """Hierarchical selection: per-128 sub-chunk maxes -> top-4 sub-chunks ->
    windowed gather from a DRAM copy of the scores -> top-8 -> fp32 rescore."""
    nc = bass.Bass("TRN2", target_bir_lowering=False, debug=False,
                   num_devices=N_CORES)

    ntok = n_tiles * TILE_TOK
    z_in = nc.dram_tensor("z", [ntok, D], F32, kind="ExternalInput").ap()
    w_in = nc.dram_tensor("w", [N_E, D], F32, kind="ExternalInput").ap()
    waug_in = nc.dram_tensor("waug", [D + 1, N_E], BF16, kind="ExternalInput").ap()
    ident_in = nc.dram_tensor("ident", [128, 128], F32, kind="ExternalInput").ap()
    iota8_in = nc.dram_tensor("iota8", [128, NCAND], F32, kind="ExternalInput").ap()
    iotab8_in = nc.dram_tensor("iotab8", [128, NCAND], F32, kind="ExternalInput").ap()
    iota4_in = nc.dram_tensor("iota4", [128, NWIN], F32, kind="ExternalInput").ap()
    prow_in = nc.dram_tensor("prow", [128, 1], F32, kind="ExternalInput").ap()

    zq_out = nc.dram_tensor("zq", [ntok, D], F32, kind="ExternalOutput").ap()
    idx_out = nc.dram_tensor("idx", [TILE_TOK, n_tiles], I32, kind="ExternalOutput").ap()
    loss_out = nc.dram_tensor("losspart", [TILE_TOK, n_tiles], F32, kind="ExternalOutput").ap()

    import os
    stub = bool(int(os.environ.get("KERNEL_STUB", "0")))

    dscr = [nc.dram_tensor(f"dscr{t}", [N_SUB * TILE_TOK, SUBW], BF16,
                           kind="Internal").ap()
            for t in range(n_tiles)]

    GROUP = 4 * CHUNK  # 2048 cols of PSUM = 4 banks per evacuation group
    N_GROUPS = N_E // GROUP

    with tile.TileContext(nc) as tc:
        with (
            tc.tile_pool(name="consts", bufs=1) as cpool,
            tc.tile_pool(name="waug", bufs=1) as wpool,
            tc.tile_pool(name="persist", bufs=1) as ppool,
            tc.tile_pool(name="acc", bufs=1) as apool,
        ):
            ident = cpool.tile([128, 128], F32, tag="ident")
            nc.sync.dma_start(ident[:], ident_in[:])
            iota8 = cpool.tile([128, NCAND], F32, tag="iota8")
            nc.sync.dma_start(iota8[:], iota8_in[:])
            iotab8 = cpool.tile([128, NCAND], F32, tag="iotab8")
            nc.sync.dma_start(iotab8[:], iotab8_in[:])
            iota4 = cpool.tile([128, NWIN], F32, tag="iota4")
            nc.sync.dma_start(iota4[:], iota4_in[:])
            prow = cpool.tile([128, 1], F32, tag="prow")
            nc.sync.dma_start(prow[:], prow_in[:])

            waug = wpool.tile([D + 1, N_E], BF16)
            nc.sync.dma_start(waug[:], waug_in[:])

            z_all = ppool.tile([TILE_TOK, n_tiles, D], F32, tag="z_all")
            nc.sync.dma_start(z_all[:],
                              z_in[:].rearrange("(t p) d -> p t d", p=TILE_TOK))
            zt_all = ppool.tile([D + 1, n_tiles * TILE_TOK], BF16, tag="zt_all")

            idx_acc = apool.tile([TILE_TOK, n_tiles], I32, tag="idx_acc")
            loss_acc = apool.tile([TILE_TOK, n_tiles], F32, tag="loss_acc")

            # ---- phase 0: transpose all z tiles into bf16 lhsT (x2) -------
            with tc.tile_pool(name="psT", bufs=4, space="PSUM") as psT:
                nc.gpsimd.memset(zt_all[D:D + 1, :], 1.0)
                for t in range(n_tiles):
                    ztp = psT.tile([D, TILE_TOK], F32)
                    nc.tensor.transpose(ztp[:], z_all[:, t, :], ident[:])
                    nc.scalar.mul(zt_all[0:D, t * TILE_TOK:(t + 1) * TILE_TOK],
                                  ztp[:], 2.0)

            # ---- main loop -----------------------------------------------
            with (
                tc.tile_pool(name="dstore", bufs=5) as dpool,
                tc.tile_pool(name="win", bufs=4) as winpool,
                tc.tile_pool(name="cand", bufs=4) as candpool,
                tc.tile_pool(name="small", bufs=6) as spool,
                tc.tile_pool(name="psMM", bufs=2, space="PSUM") as psMM,
            ):
                for t in range(n_tiles):
                    lhsT = zt_all[:, t * TILE_TOK:(t + 1) * TILE_TOK]
                    d_sb = dpool.tile([TILE_TOK, N_E], BF16)
                    for g in range(N_GROUPS):
                        mm4 = psMM.tile([TILE_TOK, GROUP], F32)
                        for k in range(4):
                            c = g * 4 + k
                            nc.tensor.matmul(
                                mm4[:, k * CHUNK:(k + 1) * CHUNK], lhsT,
                                waug[:, c * CHUNK:(c + 1) * CHUNK],
                                start=True, stop=True)
                        nc.scalar.copy(d_sb[:, g * GROUP:(g + 1) * GROUP], mm4[:])
                        # dump this group's scores to DRAM right away so the
                        # windowed gather isn't stuck behind one 2MB tail DMA
                        spg = GROUP // SUBW
                        nc.sync.dma_start(
                            dscr[t][:].rearrange("(p s) w -> p s w", p=TILE_TOK)
                            [:, g * spg:(g + 1) * spg, :],
                            d_sb[:, g * GROUP:(g + 1) * GROUP])

                    # sub-chunk maxes + top-NWIN sub-chunks
                    smax = spool.tile([TILE_TOK, N_SUB], BF16, tag="smax")
                    spg = GROUP // SUBW
                    for g in range(N_GROUPS):
                        nc.vector.tensor_reduce(
                            smax[:, g * spg:(g + 1) * spg],
                            d_sb[:, g * GROUP:(g + 1) * GROUP]
                            .rearrange("p (s w) -> p s w", w=SUBW),
                            axis=mybir.AxisListType.X, op=mybir.AluOpType.max)
                    sc8v = spool.tile([TILE_TOK, 8], BF16, tag="sc8v")
                    nc.vector.max(sc8v[:], smax[:])
                    sc8i = spool.tile([TILE_TOK, 8], U32, tag="sc8i")
                    nc.vector.max_index(sc8i[:], sc8v[:], smax[:])
                    sc_f = spool.tile([TILE_TOK, 8], F32, tag="sc_f")
                    nc.vector.tensor_copy(sc_f[:], sc8i[:])

                    if stub:
                        # timing-attribution variant: no gathers / rescore
                        nc.vector.tensor_copy(idx_acc[:, t:t + 1], sc_f[:, 0:1])
                        zqo0 = spool.tile([TILE_TOK, D], F32, tag="zqo")
                        nc.vector.tensor_copy(zqo0[:], z_all[:, t, :])
                        nc.sync.dma_start(
                            zq_out[t * TILE_TOK:(t + 1) * TILE_TOK, :], zqo0[:])
                        nc.vector.tensor_reduce(
                            loss_acc[:, t:t + 1], sc_f[:],
                            axis=mybir.AxisListType.X, op=mybir.AluOpType.add)
                        continue

                    # windowed gather of the top-NWIN sub-chunks
                    widx_f = spool.tile([TILE_TOK, NWIN], F32, tag="widx_f")
                    nc.vector.tensor_scalar_add(widx_f[:], sc_f[:, 0:NWIN], prow[:])
                    widx = spool.tile([TILE_TOK, NWIN], U32, tag="widx")
                    nc.vector.tensor_copy(widx[:], widx_f[:])
                    win = winpool.tile([TILE_TOK, NWIN, SUBW], BF16)
                    for j in range(NWIN):
                        nc.gpsimd.indirect_dma_start(
                            win[:, j, :], None, dscr[t][:],
                            IndirectOffsetOnAxis(ap=widx[:, j:j + 1], axis=0))

                    # top-8 candidates within the gathered windows
                    winflat = win[:].rearrange("p a b -> p (a b)")
                    wv8 = spool.tile([TILE_TOK, 8], BF16, tag="wv8")
                    nc.vector.max(wv8[:], winflat)
                    wi8 = spool.tile([TILE_TOK, 8], U32, tag="wi8")
                    nc.vector.max_index(wi8[:], wv8[:], winflat)
                    s_f = spool.tile([TILE_TOK, 8], F32, tag="s_f")
                    nc.vector.tensor_copy(s_f[:], wi8[:])

                    # decode window slot -> global code index
                    j8 = spool.tile([TILE_TOK, 8], F32, tag="j8")
                    nc.gpsimd.memset(j8[:], 0.0)
                    for kk in range(1, NWIN):
                        jp = spool.tile([TILE_TOK, 8], F32, tag=f"jp")
                        nc.vector.tensor_scalar(jp[:], s_f[:], float(kk * SUBW),
                                                None, op0=mybir.AluOpType.is_ge)
                        nc.vector.tensor_add(j8[:], j8[:], jp[:])
                    rem = spool.tile([TILE_TOK, 8], F32, tag="rem")
                    nc.vector.scalar_tensor_tensor(
                        rem[:], j8[:], -float(SUBW), s_f[:],
                        op0=mybir.AluOpType.mult, op1=mybir.AluOpType.add)
                    # sc_sel = sum_k (j8 == k) * sc_f[:, k]
                    eqk = spool.tile([TILE_TOK, 8, NWIN], F32, tag="eqk")
                    nc.vector.tensor_tensor(
                        eqk[:],
                        j8[:].unsqueeze(2).broadcast_to((TILE_TOK, 8, NWIN)),
                        iota4[:].unsqueeze(1).broadcast_to((TILE_TOK, 8, NWIN)),
                        mybir.AluOpType.is_equal)
                    eqs = spool.tile([TILE_TOK, 8, NWIN], F32, tag="eqs")
                    nc.vector.tensor_tensor(
                        eqs[:], eqk[:],
                        sc_f[:, 0:NWIN].unsqueeze(1).broadcast_to((TILE_TOK, 8, NWIN)),
                        mybir.AluOpType.mult)
                    sc_sel = spool.tile([TILE_TOK, 8], F32, tag="sc_sel")
                    nc.vector.tensor_reduce(sc_sel[:], eqs[:],
                                            axis=mybir.AxisListType.X,
                                            op=mybir.AluOpType.add)
                    gidx_f = spool.tile([TILE_TOK, 8], F32, tag="gidx_f")
                    nc.vector.scalar_tensor_tensor(
                        gidx_f[:], sc_sel[:], float(SUBW), rem[:],
                        op0=mybir.AluOpType.mult, op1=mybir.AluOpType.add)
                    gi32 = spool.tile([TILE_TOK, 8], U32, tag="gi32")
                    nc.vector.tensor_copy(gi32[:], gidx_f[:])

                    # gather candidate rows + fp32 rescore (top-RESC of the
                    # descending max8 scan; coverage needs top-3)
                    cand = candpool.tile([TILE_TOK, RESC, D], F32)
                    for j in range(RESC):
                        nc.gpsimd.indirect_dma_start(
                            cand[:, j, :], None, w_in[:],
                            IndirectOffsetOnAxis(ap=gi32[:, j:j + 1], axis=0))
                    zb = z_all[:, t, :].unsqueeze(1).broadcast_to((TILE_TOK, RESC, D))
                    diff = candpool.tile([TILE_TOK, RESC, D], F32, tag="diff")
                    nc.vector.tensor_tensor(diff[:], cand[:], zb,
                                            mybir.AluOpType.subtract)
                    sq = candpool.tile([TILE_TOK, RESC, D], F32, tag="sq")
                    nc.scalar.square(sq[:], diff[:])
                    s8 = spool.tile([TILE_TOK, RESC], F32, tag="s8")
                    nc.vector.tensor_reduce(s8[:], sq[:], axis=mybir.AxisListType.X,
                                            op=mybir.AluOpType.add)

                    # winner selection (first-min tie break)
                    smin = spool.tile([TILE_TOK, 1], F32, tag="smin")
                    nc.vector.tensor_reduce(smin[:], s8[:], axis=mybir.AxisListType.X,
                                            op=mybir.AluOpType.min)
                    mask8 = spool.tile([TILE_TOK, RESC], F32, tag="mask8")
                    nc.vector.tensor_scalar(mask8[:], s8[:], smin[:], None,
                                            op0=mybir.AluOpType.is_equal)
                    t8 = spool.tile([TILE_TOK, RESC], F32, tag="t8")
                    nc.vector.scalar_tensor_tensor(t8[:], mask8[:], -BIG,
                                                   iotab8[:, 0:RESC],
                                                   op0=mybir.AluOpType.mult,
                                                   op1=mybir.AluOpType.add)
                    jstar = spool.tile([TILE_TOK, 1], F32, tag="jstar")
                    nc.vector.tensor_reduce(jstar[:], t8[:], axis=mybir.AxisListType.X,
                                            op=mybir.AluOpType.min)
                    maskj = spool.tile([TILE_TOK, RESC], F32, tag="maskj")
                    nc.vector.tensor_scalar(maskj[:], iota8[:, 0:RESC], jstar[:], None,
                                            op0=mybir.AluOpType.is_equal)

                    scr8 = spool.tile([TILE_TOK, RESC], F32, tag="scr8")
                    idx_f = spool.tile([TILE_TOK, 1], F32, tag="idxf")
                    nc.vector.tensor_tensor(scr8[:], maskj[:], gidx_f[:, 0:RESC],
                                            mybir.AluOpType.mult)
                    nc.vector.tensor_reduce(idx_f[:], scr8[:],
                                            axis=mybir.AxisListType.X,
                                            op=mybir.AluOpType.add)
                    nc.vector.tensor_copy(idx_acc[:, t:t + 1], idx_f[:])

                    # z_q via one-row gather of the winner
                    iw32 = spool.tile([TILE_TOK, 1], U32, tag="iw32")
                    nc.vector.tensor_copy(iw32[:], idx_f[:])
                    zq_g = winpool.tile([TILE_TOK, 1, D], F32, tag="zq_g")
                    nc.gpsimd.indirect_dma_start(
                        zq_g[:, 0, :], None, w_in[:],
                        IndirectOffsetOnAxis(ap=iw32[:], axis=0))

                    diffw = spool.tile([TILE_TOK, D], F32, tag="diffw")
                    nc.vector.tensor_sub(diffw[:], zq_g[:, 0, :], z_all[:, t, :])
                    zqo = spool.tile([TILE_TOK, D], F32, tag="zqo")
                    nc.vector.tensor_add(zqo[:], z_all[:, t, :], diffw[:])
                    nc.sync.dma_start(zq_out[t * TILE_TOK:(t + 1) * TILE_TOK, :], zqo[:])

                    scr64 = spool.tile([TILE_TOK, D], F32, tag="scr64")
                    nc.vector.tensor_tensor(scr64[:], diffw[:], diffw[:],
                                            mybir.AluOpType.mult)
                    nc.vector.tensor_reduce(loss_acc[:, t:t + 1], scr64[:],
                                            axis=mybir.AxisListType.X,
                                            op=mybir.AluOpType.add)

                nc.sync.dma_start(idx_out[:], idx_acc[:])
                nc.sync.dma_start(loss_out[:], loss_acc[:])

    return nc


_NC_CACHE = {}


def _get_nc(n_tiles=N_TILES):
    import os
    ver = os.environ.get("KERNEL_V", "2")
    key = (ver, n_tiles)
    if key not in _NC_CACHE:
        builder = build_nc_v2 if ver == "2" else build_nc
        _NC_CACHE[key] = builder(n_tiles)
    return _NC_CACHE[key]


def make_host_inputs(z, W):
    """Host-side prep: shard z, prepack weights/constants."""
    zf = np.ascontiguousarray(z.reshape(-1, D).astype(np.float32))
    W = np.ascontiguousarray(W.astype(np.float32))

    wb = W.astype(ml_dtypes.bfloat16).astype(np.float32)
    w2 = (wb * wb).sum(axis=1, dtype=np.float32)
    waug = np.empty((D + 1, N_E), dtype=ml_dtypes.bfloat16)
    waug[:D, :] = W.T.astype(ml_dtypes.bfloat16)
    waug[D, :] = (-w2).astype(ml_dtypes.bfloat16)

    ident = np.eye(128, dtype=np.float32)
    iota8 = np.broadcast_to(np.arange(NCAND, dtype=np.float32), (128, NCAND)).copy()
    iotab8 = iota8 + BIG

    import os
    ver = os.environ.get("KERNEL_V", "2")
    in_maps = []
    for c in range(N_CORES):
        m = {
            "z": zf[c * TOK_PER_CORE:(c + 1) * TOK_PER_CORE],
            "w": W,
            "waug": waug,
            "ident": ident,
            "iota8": iota8,
            "iotab8": iotab8,
        }
        if ver == "2":
            m["iota4"] = np.broadcast_to(
                np.arange(NWIN, dtype=np.float32), (128, NWIN)).copy()
            m["prow"] = (np.arange(128, dtype=np.float32) * N_SUB).reshape(128, 1)
        in_maps.append(m)
    return in_maps


def finalize(z, results):
    """Combine per-core outputs into the reference's 4-tuple."""
    zq = np.concatenate([r["zq"] for r in results], axis=0)
    z_q_out = zq.reshape(32, 1024, D)

    idx = np.concatenate(
        [r["idx"].T.reshape(-1) for r in results], axis=0).astype(np.int32)

    sq_sum = np.float64(0.0)
    for r in results:
        sq_sum += r["losspart"].astype(np.float64).sum()
    n_elem = 32768 * D
    m = sq_sum / n_elem
    loss = np.float32((1.0 + BETA) * m)

    counts = np.bincount(idx, minlength=N_E).astype(np.float64)
    avg_probs = counts / 32768.0
    perplexity = np.float32(np.exp(-np.sum(avg_probs * np.log(avg_probs + EPS))))

    return z_q_out, loss, idx, perplexity


LAST_EXEC_NS = None


def kernel(z, W):
    global LAST_EXEC_NS
    import os
    z = np.asarray(z, dtype=np.float32)
    W = np.asarray(W, dtype=np.float32)
    nc = _get_nc()
    in_maps = make_host_inputs(z, W)
    trace = bool(int(os.environ.get("KERNEL_TRACE", "0")))
    try:
        res = run_bass_kernel_spmd(nc, in_maps, core_ids=list(range(N_CORES)),
                                   trace=trace)
    except ModuleNotFoundError:
        res = run_bass_kernel_spmd(nc, in_maps, core_ids=list(range(N_CORES)))
    if res.exec_time_ns is not None:
        LAST_EXEC_NS = res.exec_time_ns
    return finalize(z, res.results)
